# revision 20
# baseline (speedup 1.0000x reference)
"""Trainium2 Bass kernel for nn_Affinity (gnn_message_passing).

Math (per batch element b, H=128, NP=512, ND=64, DEPTH=3, N=NP*ND=32768):
  drug_feat = lrelu(drug @ d_w + d_b, .1) * du_mask ; u_d = drug_feat.sum(0)
  prot_feat = lrelu(prot @ p_w + p_b, .1) * pu_mask ; u_p = prot_feat.sum(0)
  mij[p,d]  = lrelu(prot_feat[p] @ Wv0a + drug_feat[d] @ Wv0b + Wv0_b, .1)
  m_u       = lrelu([u_d;u_p] @ Wu_w + Wu_b, .01)
  for i in 0..2:   (per pairwise row m)
    t = m @ Wv_i + Wv_b_i ; alpha = m . v_i + c_i    (v_i = Wv_i@att_i,
    g = alpha * t ; m += g                            c_i = Wv_b_i.att_i+att_b_i)
  w1in = m + g   (= m_prev + 2 g_last)
  s = sum_rows relu(w1in @ W1a + W1a_b)
  global1 = s @ W1b + N*W1b_b ; ... small MLP tail ... -> scalar per batch

Strategy: data-parallel over batch, 1 element per NeuronCore, no collectives.
Everything H-major on chip: tensors stored [H=128 partitions, rows free].
The big [N,128] intermediate lives only in SBUF, tiled FD rows at a time.
Matmuls in fp32r (full PE rate at FD>=256, ~1.5e-4 rel err), rest fp32.
"""

import os
import sys

if "/opt/trn_rl_repo" not in sys.path:
    sys.path.insert(0, "/opt/trn_rl_repo")

import numpy as np
from contextlib import ExitStack

import concourse.bass as bass
import concourse.tile as tile
from concourse import bacc, mybir
from concourse.bass_utils import run_bass_kernel_spmd

F32 = mybir.dt.float32
F32R = mybir.dt.float32r
AF = mybir.ActivationFunctionType
OP = mybir.AluOpType

B, NP, ND, H, DEPTH = 8, 512, 64, 128, 3
N = NP * ND
FD = 512           # rows per macro tile (TD=FD//NP d-values per tile)
TD = FD // NP      # d-values per macro tile
NT = N // FD       # macro tiles per core

# weight slot order in the packed [128, 128*NW] weight matrix
W_NAMES = [
    "p_w", "d_w", "Wv0a", "Wv0b", "Wv_0", "Wv_1", "Wv_2",
    "vrep_0", "vrep_1", "vrep_2", "W1a",
    "W1b", "W2a_0", "W2a_1", "W2b_0", "W2b_1", "W3",
    "Wu_0", "Wu_1", "Wu1a_0", "Wu1a_1", "Wu1b",
]
WIDX = {n: i for i, n in enumerate(W_NAMES)}
NW = len(W_NAMES)

B_NAMES = [
    "p_b", "d_b", "Wv0_b", "Wv_b0", "Wv_b1", "Wv_b2", "c0", "c1", "c2",
    "Wu_b", "W1a_b", "W1b_bN", "W2a_b0", "W2a_b1", "W2b_b", "W3_b",
    "Wu1a_b", "Wu1b_b", "W5_b", "W5_w",
]
BIDX = {n: i for i, n in enumerate(B_NAMES)}
NB = len(B_NAMES)


def build_program():
    import os as _os
    nc = bacc.Bacc("TRN2", target_bir_lowering=False, debug=False, num_devices=8)

    pfT_d = nc.dram_tensor("pfT", [H, NP], F32, kind="ExternalInput")
    dfT_d = nc.dram_tensor("dfT", [H, ND], F32, kind="ExternalInput")
    pmask_d = nc.dram_tensor("pmask", [H, NP], F32, kind="ExternalInput")
    dmask_d = nc.dram_tensor("dmask", [H, ND], F32, kind="ExternalInput")
    wmat_d = nc.dram_tensor("wmat", [H, 128 * NW], F32, kind="ExternalInput")
    bmat_d = nc.dram_tensor("bmat", [H, NB], F32, kind="ExternalInput")
    out_d = nc.dram_tensor("out", [1, 1], F32, kind="ExternalOutput")

    def w(ap, name):
        i = WIDX[name]
        return ap[:, i * 128:(i + 1) * 128]

    def bcol(ap, name):
        i = BIDX[name]
        return ap[:, i:i + 1]

    with tile.TileContext(nc) as tc, ExitStack() as ctx:
        sbw = ctx.enter_context(tc.tile_pool(name="sbw", bufs=1))

        wsb = sbw.tile([H, 128 * NW], F32, tag="wsb")
        nc.sync.dma_start(out=wsb[:], in_=wmat_d[:])
        bsb = sbw.tile([H, NB], F32, tag="bsb")
        nc.sync.dma_start(out=bsb[:], in_=bmat_d[:])
        pfT = sbw.tile([H, NP], F32, tag="pfT")
        nc.sync.dma_start(out=pfT[:], in_=pfT_d[:])
        dfT = sbw.tile([H, ND], F32, tag="dfT")
        nc.sync.dma_start(out=dfT[:], in_=dfT_d[:])
        pmask = sbw.tile([H, NP], F32, tag="pmask")
        nc.sync.dma_start(out=pmask[:], in_=pmask_d[:])
        dmask = sbw.tile([H, ND], F32, tag="dmask")
        nc.sync.dma_start(out=dmask[:], in_=dmask_d[:])

        # fp32r copies of the per-row-GEMM weights
        wr = {}
        for name in ["Wv_0", "Wv_1", "Wv_2", "vrep_0", "vrep_1", "vrep_2",
                     "W1a"]:
            t = sbw.tile([H, 128], F32R, tag=f"r_{name}")
            nc.vector.tensor_copy(t[:], w(wsb, name))
            wr[name] = t

        # ---- startup: projections, u_d/u_p, m_u, prot/drug proj ----
        ppT = sbw.tile([H, NP], F32, tag="ppT")
        dpT = sbw.tile([H, ND], F32, tag="dpT")
        u_p = sbw.tile([H, 1], F32, tag="u_p")
        u_d = sbw.tile([H, 1], F32, tag="u_d")
        mu_t = sbw.tile([H, 1], F32, tag="mu_t")
        acc = sbw.tile([H, NT], F32, tag="acc")

        with tc.tile_pool(name="ps0", bufs=2, space="PSUM") as ps0:
            # prot: feat = prelu(p_w.T @ pfT + p_b); masked + u_p
            ps_pf = ps0.tile([H, NP], F32, tag="ps0")
            nc.tensor.matmul(ps_pf[:], w(wsb, "p_w"), pfT[:], start=True, stop=True)
            pf_act = sbw.tile([H, NP], F32, tag="pf_act")
            nc.scalar.activation(pf_act[:], ps_pf[:], AF.Prelu,
                                 bias=bcol(bsb, "p_b"), scale=1.0, alpha=0.1)
            pf_m = sbw.tile([H, NP], F32, tag="pf_m")
            nc.vector.scalar_tensor_tensor(pf_m[:], pf_act[:], 1.0, pmask[:],
                                           op0=OP.mult, op1=OP.mult,
                                           accum_out=u_p[:])
            # drug
            ps_df = ps0.tile([H, ND], F32, tag="ps0")
            nc.tensor.matmul(ps_df[:], w(wsb, "d_w"), dfT[:], start=True, stop=True)
            df_act = sbw.tile([H, ND], F32, tag="df_act")
            nc.scalar.activation(df_act[:], ps_df[:], AF.Prelu,
                                 bias=bcol(bsb, "d_b"), scale=1.0, alpha=0.1)
            df_m = sbw.tile([H, ND], F32, tag="df_m")
            nc.vector.scalar_tensor_tensor(df_m[:], df_act[:], 1.0, dmask[:],
                                           op0=OP.mult, op1=OP.mult,
                                           accum_out=u_d[:])
            # pairwise projections (Wv0_b folded into prot side)
            ps_pp = ps0.tile([H, NP], F32, tag="ps0")
            nc.tensor.matmul(ps_pp[:], w(wsb, "Wv0a"), pf_m[:], start=True, stop=True)
            nc.scalar.activation(ppT[:], ps_pp[:], AF.Identity,
                                 bias=bcol(bsb, "Wv0_b"), scale=1.0)
            ps_dp = ps0.tile([H, ND], F32, tag="ps0")
            nc.tensor.matmul(ps_dp[:], w(wsb, "Wv0b"), df_m[:], start=True, stop=True)
            nc.scalar.copy(dpT[:], ps_dp[:])
            # m_u = prelu(Wu.T @ [u_d;u_p] + Wu_b, .01)
            ps_mu = ps0.tile([H, 1], F32, tag="ps0s")
            nc.tensor.matmul(ps_mu[:], w(wsb, "Wu_0"), u_d[:], start=True, stop=False)
            nc.tensor.matmul(ps_mu[:], w(wsb, "Wu_1"), u_p[:], start=False, stop=True)
            nc.scalar.activation(mu_t[:], ps_mu[:], AF.Prelu,
                                 bias=bcol(bsb, "Wu_b"), scale=1.0, alpha=0.01)

        # ---- main loop over macro tiles (software-pipelined emission) ----
        # Stages per tile t:
        #   S0: construct m0        S1/S2/S3: depth 0/1/2        S4: W1a+relu
        # Emitted deepest-stage-first per step so each engine's instruction
        # stream interleaves stages of staggered tiles (no cross-tile stalls).
        with tc.tile_pool(name="sbm", bufs=16) as sbm, \
             tc.tile_pool(name="sbg", bufs=12) as sbg, \
             tc.tile_pool(name="pst", bufs=3, space="PSUM") as pst, \
             tc.tile_pool(name="psa", bufs=3, space="PSUM") as psa, \
             tc.tile_pool(name="psw", bufs=2, space="PSUM") as psw:
            NTL = NT
            # test.py wraps the main loop in an on-device For_i for timing
            reps = int(_os.environ.get("K_REPS", "0"))
            mset, g2set = {}, {}

            def s0(t):
                m = sbm.tile([H, FD], F32R, tag="m")
                nc.scalar.activation(m[:], ppT[:], AF.Prelu,
                                     bias=dpT[:, t:t + 1], scale=1.0, alpha=0.1)
                mset[t] = m

            def depth(t, i):
                m = mset[t]
                pa = psa.tile([H, FD], F32, tag="pa")
                pt = pst.tile([H, FD], F32, tag="pt")
                nc.tensor.matmul(pa[:], wr[f"vrep_{i}"][:], m[:],
                                 start=True, stop=True)
                nc.tensor.matmul(pt[:], wr[f"Wv_{i}"][:], m[:],
                                 start=True, stop=True)
                al = sbg.tile([H, FD], F32, tag="al")
                scl = 2.0 if i == 2 else 1.0  # fold g2' = 2*g2 into alpha2
                if (3 * t + i) % 4 == 0:
                    if i == 2:
                        nc.vector.tensor_scalar(al[:], pa[:],
                                                bcol(bsb, f"c{i}"), 2.0,
                                                op0=OP.add, op1=OP.mult)
                    else:
                        nc.vector.tensor_scalar_add(al[:], pa[:],
                                                    bcol(bsb, f"c{i}"))
                else:
                    nc.scalar.activation(al[:], pa[:], AF.Identity,
                                         bias=bcol(bsb, f"c{i}"), scale=scl)
                g = sbg.tile([H, FD], F32R if i == 2 else F32, tag="g")
                nc.vector.scalar_tensor_tensor(g[:], pt[:],
                                               bcol(bsb, f"Wv_b{i}"), al[:],
                                               op0=OP.add, op1=OP.mult)
                if i < 2:
                    m2 = sbm.tile([H, FD], F32R, tag="m")
                    nc.gpsimd.tensor_tensor(m2[:], mset.pop(t)[:].bitcast(F32),
                                            g[:], op=OP.add)
                    mset[t] = m2
                else:
                    g2set[t] = g

            def s4(t):
                m2, g2 = mset.pop(t), g2set.pop(t)
                pw = psw.tile([H, FD], F32, tag="pw")
                nc.tensor.matmul(pw[:], wr["W1a"][:], m2[:],
                                 start=True, stop=False)
                nc.tensor.matmul(pw[:], wr["W1a"][:], g2[:],
                                 start=False, stop=True)
                scr = sbg.tile([H, FD], F32, tag="scr")
                nc.scalar.activation(scr[:], pw[:], AF.Relu,
                                     bias=bcol(bsb, "W1a_b"), scale=1.0,
                                     accum_out=acc[:, t:t + 1])

            LAG = 3

            def main_loop():
                for step in range(NTL + 4 * LAG):
                    if step >= 4 * LAG and step - 4 * LAG < NTL:
                        s4(step - 4 * LAG)
                    if step >= 3 * LAG and step - 3 * LAG < NTL:
                        depth(step - 3 * LAG, 2)
                    if step >= 2 * LAG and step - 2 * LAG < NTL:
                        depth(step - 2 * LAG, 1)
                    if step >= LAG and step - LAG < NTL:
                        depth(step - LAG, 0)
                    if step < NTL:
                        s0(step)

            if reps:
                with tc.For_i(0, reps, 1,
                              hint_engines=(mybir.EngineType.PE,
                                            mybir.EngineType.Activation,
                                            mybir.EngineType.DVE,
                                            mybir.EngineType.Pool,
                                            mybir.EngineType.SP)):
                    main_loop()
            else:
                main_loop()


        # ---- tail MLP (tiny) ----
        with tc.tile_pool(name="pse", bufs=2, space="PSUM") as pse, \
             tc.tile_pool(name="sbe", bufs=1) as sbe:
            s_t = sbe.tile([H, 1], F32, tag="s")
            nc.vector.tensor_reduce(s_t[:], acc[:], axis=mybir.AxisListType.X,
                                    op=OP.add)
            p1 = pse.tile([H, 1], F32, tag="pse")
            nc.tensor.matmul(p1[:], w(wsb, "W1b"), s_t[:], start=True, stop=True)
            t1 = sbe.tile([H, 1], F32, tag="t1")
            nc.scalar.activation(t1[:], p1[:], AF.Identity,
                                 bias=bcol(bsb, "W1b_bN"), scale=1.0)
            p2a = pse.tile([H, 2], F32, tag="pse")
            nc.tensor.matmul(p2a[:, 0:1], w(wsb, "W2a_0"), t1[:], start=True, stop=True)
            nc.tensor.matmul(p2a[:, 1:2], w(wsb, "W2a_1"), t1[:], start=True, stop=True)
            t2a = sbe.tile([H, 2], F32, tag="t2a")
            nc.scalar.activation(t2a[:, 0:1], p2a[:, 0:1], AF.Prelu,
                                 bias=bcol(bsb, "W2a_b0"), scale=1.0, alpha=0.1)
            nc.scalar.activation(t2a[:, 1:2], p2a[:, 1:2], AF.Prelu,
                                 bias=bcol(bsb, "W2a_b1"), scale=1.0, alpha=0.1)
            p2 = pse.tile([H, 1], F32, tag="pse")
            nc.tensor.matmul(p2[:], w(wsb, "W2b_0"), t2a[:, 0:1], start=True, stop=False)
            nc.tensor.matmul(p2[:], w(wsb, "W2b_1"), t2a[:, 1:2], start=False, stop=True)
            t2 = sbe.tile([H, 1], F32, tag="t2")
            nc.scalar.activation(t2[:], p2[:], AF.Identity,
                                 bias=bcol(bsb, "W2b_b"), scale=1.0)
            p3 = pse.tile([H, 1], F32, tag="pse")
            nc.tensor.matmul(p3[:], w(wsb, "W3"), t2[:], start=True, stop=True)
            t3 = sbe.tile([H, 1], F32, tag="t3")
            nc.scalar.activation(t3[:], p3[:], AF.Prelu,
                                 bias=bcol(bsb, "W3_b"), scale=1.0, alpha=0.1)
            p4 = pse.tile([H, 1], F32, tag="pse")
            nc.tensor.matmul(p4[:], w(wsb, "Wu1a_0"), mu_t[:], start=True, stop=False)
            nc.tensor.matmul(p4[:], w(wsb, "Wu1a_1"), t3[:], start=False, stop=True)
            t4 = sbe.tile([H, 1], F32, tag="t4")
            nc.scalar.activation(t4[:], p4[:], AF.Prelu,
                                 bias=bcol(bsb, "Wu1a_b"), scale=1.0, alpha=0.1)
            p5 = pse.tile([H, 1], F32, tag="pse")
            nc.tensor.matmul(p5[:], w(wsb, "Wu1b"), t4[:], start=True, stop=True)
            t5 = sbe.tile([H, 1], F32, tag="t5")
            nc.scalar.activation(t5[:], p5[:], AF.Identity,
                                 bias=bcol(bsb, "Wu1b_b"), scale=1.0)
            p6 = pse.tile([1, 1], F32, tag="pse")
            nc.tensor.matmul(p6[:], bcol(bsb, "W5_w"), t5[:], start=True, stop=True)
            o_sb = sbe.tile([1, 1], F32, tag="o")
            nc.scalar.activation(o_sb[:], p6[:], AF.Identity,
                                 bias=bsb[0:1, BIDX["W5_b"]:BIDX["W5_b"] + 1],
                                 scale=1.0)
            nc.sync.dma_start(out=out_d[:], in_=o_sb[:])

    nc.compile()
    return nc


_NC = None


def _get_nc():
    global _NC
    if _NC is None:
        _NC = build_program()
    return _NC


def _host_prep(inputs):
    """Build per-core in_maps from full inputs (weight transforms on host)."""
    f = {k: np.asarray(v, dtype=np.float32) for k, v in inputs.items()}

    # v_i = Wv_w[i] @ att_w[i],  c_i = att_w[i].Wv_b[i] + att_b[i]
    Wv_w, att_w = f["Wv_w"], f["att_w"]
    Wv_b, att_b = f["Wv_b"], f["att_b"]
    vs = [Wv_w[i] @ att_w[i] for i in range(DEPTH)]
    cs = [float(att_w[i] @ Wv_b[i] + att_b[i]) for i in range(DEPTH)]

    wcols = {
        "p_w": f["p_w"], "d_w": f["d_w"],
        "Wv0a": f["Wv0_w"][:H], "Wv0b": f["Wv0_w"][H:],
        "Wv_0": Wv_w[0], "Wv_1": Wv_w[1], "Wv_2": Wv_w[2],
        "vrep_0": np.repeat(vs[0][:, None], 128, 1),
        "vrep_1": np.repeat(vs[1][:, None], 128, 1),
        "vrep_2": np.repeat(vs[2][:, None], 128, 1),
        "W1a": f["W1a_w"],
        "W1b": f["W1b_w"],
        "W2a_0": f["W2a_w"][:, :H], "W2a_1": f["W2a_w"][:, H:],
        "W2b_0": f["W2b_w"][:H], "W2b_1": f["W2b_w"][H:],
        "W3": f["W3_w"],
        "Wu_0": f["Wu_w"][:H], "Wu_1": f["Wu_w"][H:],
        "Wu1a_0": f["Wu1a_w"][:H], "Wu1a_1": f["Wu1a_w"][H:],
        "Wu1b": f["Wu1b_w"],
    }
    wmat = np.concatenate([np.ascontiguousarray(wcols[n]) for n in W_NAMES],
                          axis=1)

    def bc(v):
        v = np.asarray(v, dtype=np.float32).reshape(-1)
        if v.size == 1:
            return np.full((H,), float(v[0]), dtype=np.float32)
        assert v.size == H
        return v

    bvals = {
        "p_b": f["p_b"], "d_b": f["d_b"], "Wv0_b": f["Wv0_b"],
        "Wv_b0": Wv_b[0], "Wv_b1": Wv_b[1], "Wv_b2": Wv_b[2],
        "c0": cs[0], "c1": cs[1], "c2": cs[2],
        "Wu_b": f["Wu_b"], "W1a_b": f["W1a_b"],
        "W1b_bN": f["W1b_b"] * np.float32(N),
        "W2a_b0": f["W2a_b"][:H], "W2a_b1": f["W2a_b"][H:],
        "W2b_b": f["W2b_b"], "W3_b": f["W3_b"],
        "Wu1a_b": f["Wu1a_b"], "Wu1b_b": f["Wu1b_b"],
        "W5_b": f["W5_b"], "W5_w": f["W5_w"][:, 0],
    }
    bmat = np.stack([bc(bvals[n]) for n in B_NAMES], axis=1)
    bmat = np.ascontiguousarray(bmat, dtype=np.float32)

    pf, df = f["protein_features"], f["drug_features"]
    pm, dm = f["pu_mask"], f["du_mask"]
    in_maps = []
    for b in range(B):
        in_maps.append({
            "pfT": np.ascontiguousarray(pf[b].T),
            "dfT": np.ascontiguousarray(df[b].T),
            "pmask": np.ascontiguousarray(
                np.broadcast_to(pm[b][None, :], (H, NP))),
            "dmask": np.ascontiguousarray(
                np.broadcast_to(dm[b][None, :], (H, ND))),
            "wmat": wmat,
            "bmat": bmat,
        })
    return in_maps


def kernel(**inputs) -> np.ndarray:
    nc = _get_nc()
    in_maps = _host_prep(inputs)
    res = run_bass_kernel_spmd(nc, in_maps, list(range(B)))
    out = np.concatenate([res.results[b]["out"] for b in range(B)], axis=0)
    return out.astype(np.float32).reshape(B, 1)


# revision 21
# speedup vs baseline: 1.2131x; 1.2131x over previous
"""Trainium2 Bass kernel for nn_Affinity (gnn_message_passing).

Math (per batch element b, H=128, NP=512, ND=64, DEPTH=3, N=NP*ND=32768):
  drug_feat = lrelu(drug @ d_w + d_b, .1) * du_mask ; u_d = drug_feat.sum(0)
  prot_feat = lrelu(prot @ p_w + p_b, .1) * pu_mask ; u_p = prot_feat.sum(0)
  mij[p,d]  = lrelu(prot_feat[p] @ Wv0a + drug_feat[d] @ Wv0b + Wv0_b, .1)
  m_u       = lrelu([u_d;u_p] @ Wu_w + Wu_b, .01)
  for i in 0..2:   (per pairwise row m)
    t = m @ Wv_i + Wv_b_i ; alpha = m . v_i + c_i    (v_i = Wv_i@att_i,
    g = alpha * t ; m += g                            c_i = Wv_b_i.att_i+att_b_i)
  w1in = m + g   (= m_prev + 2 g_last)
  s = sum_rows relu(w1in @ W1a + W1a_b)
  global1 = s @ W1b + N*W1b_b ; ... small MLP tail ... -> scalar per batch

Strategy: data-parallel over batch, 1 element per NeuronCore, no collectives.
Everything H-major on chip: tensors stored [H=128 partitions, rows free].
The big [N,128] intermediate lives only in SBUF, tiled FD rows at a time.
Matmuls in fp32r (full PE rate at FD>=256, ~1.5e-4 rel err), rest fp32.
"""

import os
import sys

if "/opt/trn_rl_repo" not in sys.path:
    sys.path.insert(0, "/opt/trn_rl_repo")

import numpy as np
from contextlib import ExitStack

import concourse.bass as bass
import concourse.tile as tile
from concourse import bacc, mybir
from concourse.bass_utils import run_bass_kernel_spmd

F32 = mybir.dt.float32
F32R = mybir.dt.float32r
AF = mybir.ActivationFunctionType
OP = mybir.AluOpType

B, NP, ND, H, DEPTH = 8, 512, 64, 128, 3
N = NP * ND
FD = 512           # rows per macro tile (TD=FD//NP d-values per tile)
TD = FD // NP      # d-values per macro tile
NT = N // FD       # macro tiles per core

# weight slot order in the packed [128, 128*NW] weight matrix
W_NAMES = [
    "p_w", "d_w", "Wv0a", "Wv0b", "Wv_0", "Wv_1", "Wv_2",
    "vrep_0", "vrep_1", "vrep_2", "W1a",
    "W1b", "W2a_0", "W2a_1", "W2b_0", "W2b_1", "W3",
    "Wu_0", "Wu_1", "Wu1a_0", "Wu1a_1", "Wu1b",
]
WIDX = {n: i for i, n in enumerate(W_NAMES)}
NW = len(W_NAMES)

B_NAMES = [
    "p_b", "d_b", "Wv0_b", "Wv_b0", "Wv_b1", "Wv_b2", "c0", "c1", "c2",
    "Wu_b", "W1a_b", "W1b_bN", "W2a_b0", "W2a_b1", "W2b_b", "W3_b",
    "Wu1a_b", "Wu1b_b", "W5_b", "W5_w",
]
BIDX = {n: i for i, n in enumerate(B_NAMES)}
NB = len(B_NAMES)


def build_program():
    import os as _os
    nc = bacc.Bacc("TRN2", target_bir_lowering=False, debug=False, num_devices=8)

    pfT_d = nc.dram_tensor("pfT", [H, NP], F32, kind="ExternalInput")
    dfT_d = nc.dram_tensor("dfT", [H, ND], F32, kind="ExternalInput")
    pmask_d = nc.dram_tensor("pmask", [H, NP], F32, kind="ExternalInput")
    dmask_d = nc.dram_tensor("dmask", [H, ND], F32, kind="ExternalInput")
    wmat_d = nc.dram_tensor("wmat", [H, 128 * NW], F32, kind="ExternalInput")
    bmat_d = nc.dram_tensor("bmat", [H, NB], F32, kind="ExternalInput")
    out_d = nc.dram_tensor("out", [1, 1], F32, kind="ExternalOutput")

    def w(ap, name):
        i = WIDX[name]
        return ap[:, i * 128:(i + 1) * 128]

    def bcol(ap, name):
        i = BIDX[name]
        return ap[:, i:i + 1]

    with tile.TileContext(nc) as tc, ExitStack() as ctx:
        sbw = ctx.enter_context(tc.tile_pool(name="sbw", bufs=1))

        wsb = sbw.tile([H, 128 * NW], F32, tag="wsb")
        for c0 in range(0, 128 * NW, 128 * 6):
            c1 = min(c0 + 128 * 6, 128 * NW)
            nc.sync.dma_start(out=wsb[:, c0:c1], in_=wmat_d[:, c0:c1])
        bsb = sbw.tile([H, NB], F32, tag="bsb")
        nc.sync.dma_start(out=bsb[:], in_=bmat_d[:])
        pfT = sbw.tile([H, NP], F32, tag="pfT")
        nc.sync.dma_start(out=pfT[:], in_=pfT_d[:])
        dfT = sbw.tile([H, ND], F32, tag="dfT")
        nc.sync.dma_start(out=dfT[:], in_=dfT_d[:])
        pmask = sbw.tile([H, NP], F32, tag="pmask")
        nc.sync.dma_start(out=pmask[:], in_=pmask_d[:])
        dmask = sbw.tile([H, ND], F32, tag="dmask")
        nc.sync.dma_start(out=dmask[:], in_=dmask_d[:])

        # fp32r copies of the per-row-GEMM weights
        wr = {}
        for name in ["Wv_0", "Wv_1", "Wv_2", "vrep_0", "vrep_1", "vrep_2",
                     "W1a"]:
            t = sbw.tile([H, 128], F32R, tag=f"r_{name}")
            nc.vector.tensor_copy(t[:], w(wsb, name))
            wr[name] = t

        # ---- startup: projections, u_d/u_p, m_u, prot/drug proj ----
        ppT = sbw.tile([H, NP], F32, tag="ppT")
        dpT = sbw.tile([H, ND], F32, tag="dpT")
        u_p = sbw.tile([H, 1], F32, tag="u_p")
        u_d = sbw.tile([H, 1], F32, tag="u_d")
        mu_t = sbw.tile([H, 1], F32, tag="mu_t")
        acc = sbw.tile([H, NT], F32, tag="acc")

        with tc.tile_pool(name="ps0", bufs=2, space="PSUM") as ps0:
            # prot: feat = prelu(p_w.T @ pfT + p_b); masked + u_p
            ps_pf = ps0.tile([H, NP], F32, tag="ps0")
            nc.tensor.matmul(ps_pf[:], w(wsb, "p_w"), pfT[:], start=True, stop=True)
            pf_act = sbw.tile([H, NP], F32, tag="pf_act")
            nc.scalar.activation(pf_act[:], ps_pf[:], AF.Prelu,
                                 bias=bcol(bsb, "p_b"), scale=1.0, alpha=0.1)
            pf_m = sbw.tile([H, NP], F32, tag="pf_m")
            nc.vector.scalar_tensor_tensor(pf_m[:], pf_act[:], 1.0, pmask[:],
                                           op0=OP.mult, op1=OP.mult,
                                           accum_out=u_p[:])
            # drug
            ps_df = ps0.tile([H, ND], F32, tag="ps0")
            nc.tensor.matmul(ps_df[:], w(wsb, "d_w"), dfT[:], start=True, stop=True)
            df_act = sbw.tile([H, ND], F32, tag="df_act")
            nc.scalar.activation(df_act[:], ps_df[:], AF.Prelu,
                                 bias=bcol(bsb, "d_b"), scale=1.0, alpha=0.1)
            df_m = sbw.tile([H, ND], F32, tag="df_m")
            nc.vector.scalar_tensor_tensor(df_m[:], df_act[:], 1.0, dmask[:],
                                           op0=OP.mult, op1=OP.mult,
                                           accum_out=u_d[:])
            # pairwise projections (Wv0_b folded into prot side)
            ps_pp = ps0.tile([H, NP], F32, tag="ps0")
            nc.tensor.matmul(ps_pp[:], w(wsb, "Wv0a"), pf_m[:], start=True, stop=True)
            nc.scalar.activation(ppT[:], ps_pp[:], AF.Identity,
                                 bias=bcol(bsb, "Wv0_b"), scale=1.0)
            ps_dp = ps0.tile([H, ND], F32, tag="ps0")
            nc.tensor.matmul(ps_dp[:], w(wsb, "Wv0b"), df_m[:], start=True, stop=True)
            nc.scalar.copy(dpT[:], ps_dp[:])
            # m_u = prelu(Wu.T @ [u_d;u_p] + Wu_b, .01)
            ps_mu = ps0.tile([H, 1], F32, tag="ps0s")
            nc.tensor.matmul(ps_mu[:], w(wsb, "Wu_0"), u_d[:], start=True, stop=False)
            nc.tensor.matmul(ps_mu[:], w(wsb, "Wu_1"), u_p[:], start=False, stop=True)
            nc.scalar.activation(mu_t[:], ps_mu[:], AF.Prelu,
                                 bias=bcol(bsb, "Wu_b"), scale=1.0, alpha=0.01)

        # ---- main loop over macro tiles (software-pipelined emission) ----
        # Stages per tile t:
        #   S0: construct m0        S1/S2/S3: depth 0/1/2        S4: W1a+relu
        # Emitted deepest-stage-first per step so each engine's instruction
        # stream interleaves stages of staggered tiles (no cross-tile stalls).
        with tc.tile_pool(name="sbm", bufs=16) as sbm, \
             tc.tile_pool(name="sbg", bufs=12) as sbg, \
             tc.tile_pool(name="pst", bufs=3, space="PSUM") as pst, \
             tc.tile_pool(name="psa", bufs=3, space="PSUM") as psa, \
             tc.tile_pool(name="psw", bufs=2, space="PSUM") as psw:
            NTL = NT
            # test.py wraps the main loop in an on-device For_i for timing
            reps = int(_os.environ.get("K_REPS", "0"))
            mset, g2set = {}, {}

            def s0(t):
                m = sbm.tile([H, FD], F32R, tag="m")
                nc.scalar.activation(m[:], ppT[:], AF.Prelu,
                                     bias=dpT[:, t:t + 1], scale=1.0, alpha=0.1)
                mset[t] = m

            def depth(t, i):
                m = mset[t]
                pa = psa.tile([H, FD], F32, tag="pa")
                pt = pst.tile([H, FD], F32, tag="pt")
                nc.tensor.matmul(pa[:], wr[f"vrep_{i}"][:], m[:],
                                 start=True, stop=True)
                nc.tensor.matmul(pt[:], wr[f"Wv_{i}"][:], m[:],
                                 start=True, stop=True)
                al = sbg.tile([H, FD], F32, tag="al")
                scl = 2.0 if i == 2 else 1.0  # fold g2' = 2*g2 into alpha2
                if (3 * t + i) % 4 == 0:
                    if i == 2:
                        nc.vector.tensor_scalar(al[:], pa[:],
                                                bcol(bsb, f"c{i}"), 2.0,
                                                op0=OP.add, op1=OP.mult)
                    else:
                        nc.vector.tensor_scalar_add(al[:], pa[:],
                                                    bcol(bsb, f"c{i}"))
                else:
                    nc.scalar.activation(al[:], pa[:], AF.Identity,
                                         bias=bcol(bsb, f"c{i}"), scale=scl)
                g = sbg.tile([H, FD], F32R if i == 2 else F32, tag="g")
                nc.vector.scalar_tensor_tensor(g[:], pt[:],
                                               bcol(bsb, f"Wv_b{i}"), al[:],
                                               op0=OP.add, op1=OP.mult)
                if i < 2:
                    m2 = sbm.tile([H, FD], F32R, tag="m")
                    nc.gpsimd.tensor_tensor(m2[:], mset.pop(t)[:].bitcast(F32),
                                            g[:], op=OP.add)
                    mset[t] = m2
                else:
                    g2set[t] = g

            def s4(t):
                m2, g2 = mset.pop(t), g2set.pop(t)
                pw = psw.tile([H, FD], F32, tag="pw")
                nc.tensor.matmul(pw[:], wr["W1a"][:], m2[:],
                                 start=True, stop=False)
                nc.tensor.matmul(pw[:], wr["W1a"][:], g2[:],
                                 start=False, stop=True)
                scr = sbg.tile([H, FD], F32, tag="scr")
                nc.scalar.activation(scr[:], pw[:], AF.Relu,
                                     bias=bcol(bsb, "W1a_b"), scale=1.0,
                                     accum_out=acc[:, t:t + 1])

            LAG = 3

            def main_loop():
                for step in range(NTL + 4 * LAG):
                    if step >= 4 * LAG and step - 4 * LAG < NTL:
                        s4(step - 4 * LAG)
                    if step >= 3 * LAG and step - 3 * LAG < NTL:
                        depth(step - 3 * LAG, 2)
                    if step >= 2 * LAG and step - 2 * LAG < NTL:
                        depth(step - 2 * LAG, 1)
                    if step >= LAG and step - LAG < NTL:
                        depth(step - LAG, 0)
                    if step < NTL:
                        s0(step)

            if reps:
                with tc.For_i(0, reps, 1,
                              hint_engines=(mybir.EngineType.PE,
                                            mybir.EngineType.Activation,
                                            mybir.EngineType.DVE,
                                            mybir.EngineType.Pool,
                                            mybir.EngineType.SP)):
                    main_loop()
            else:
                main_loop()


        # ---- tail MLP (tiny) ----
        with tc.tile_pool(name="pse", bufs=2, space="PSUM") as pse, \
             tc.tile_pool(name="sbe", bufs=1) as sbe:
            s_t = sbe.tile([H, 1], F32, tag="s")
            nc.vector.tensor_reduce(s_t[:], acc[:], axis=mybir.AxisListType.X,
                                    op=OP.add)
            p1 = pse.tile([H, 1], F32, tag="pse")
            nc.tensor.matmul(p1[:], w(wsb, "W1b"), s_t[:], start=True, stop=True)
            t1 = sbe.tile([H, 1], F32, tag="t1")
            nc.scalar.activation(t1[:], p1[:], AF.Identity,
                                 bias=bcol(bsb, "W1b_bN"), scale=1.0)
            p2a = pse.tile([H, 2], F32, tag="pse")
            nc.tensor.matmul(p2a[:, 0:1], w(wsb, "W2a_0"), t1[:], start=True, stop=True)
            nc.tensor.matmul(p2a[:, 1:2], w(wsb, "W2a_1"), t1[:], start=True, stop=True)
            t2a = sbe.tile([H, 2], F32, tag="t2a")
            nc.scalar.activation(t2a[:, 0:1], p2a[:, 0:1], AF.Prelu,
                                 bias=bcol(bsb, "W2a_b0"), scale=1.0, alpha=0.1)
            nc.scalar.activation(t2a[:, 1:2], p2a[:, 1:2], AF.Prelu,
                                 bias=bcol(bsb, "W2a_b1"), scale=1.0, alpha=0.1)
            p2 = pse.tile([H, 1], F32, tag="pse")
            nc.tensor.matmul(p2[:], w(wsb, "W2b_0"), t2a[:, 0:1], start=True, stop=False)
            nc.tensor.matmul(p2[:], w(wsb, "W2b_1"), t2a[:, 1:2], start=False, stop=True)
            t2 = sbe.tile([H, 1], F32, tag="t2")
            nc.scalar.activation(t2[:], p2[:], AF.Identity,
                                 bias=bcol(bsb, "W2b_b"), scale=1.0)
            p3 = pse.tile([H, 1], F32, tag="pse")
            nc.tensor.matmul(p3[:], w(wsb, "W3"), t2[:], start=True, stop=True)
            t3 = sbe.tile([H, 1], F32, tag="t3")
            nc.scalar.activation(t3[:], p3[:], AF.Prelu,
                                 bias=bcol(bsb, "W3_b"), scale=1.0, alpha=0.1)
            p4 = pse.tile([H, 1], F32, tag="pse")
            nc.tensor.matmul(p4[:], w(wsb, "Wu1a_0"), mu_t[:], start=True, stop=False)
            nc.tensor.matmul(p4[:], w(wsb, "Wu1a_1"), t3[:], start=False, stop=True)
            t4 = sbe.tile([H, 1], F32, tag="t4")
            nc.scalar.activation(t4[:], p4[:], AF.Prelu,
                                 bias=bcol(bsb, "Wu1a_b"), scale=1.0, alpha=0.1)
            p5 = pse.tile([H, 1], F32, tag="pse")
            nc.tensor.matmul(p5[:], w(wsb, "Wu1b"), t4[:], start=True, stop=True)
            t5 = sbe.tile([H, 1], F32, tag="t5")
            nc.scalar.activation(t5[:], p5[:], AF.Identity,
                                 bias=bcol(bsb, "Wu1b_b"), scale=1.0)
            p6 = pse.tile([1, 1], F32, tag="pse")
            nc.tensor.matmul(p6[:], bcol(bsb, "W5_w"), t5[:], start=True, stop=True)
            o_sb = sbe.tile([1, 1], F32, tag="o")
            nc.scalar.activation(o_sb[:], p6[:], AF.Identity,
                                 bias=bsb[0:1, BIDX["W5_b"]:BIDX["W5_b"] + 1],
                                 scale=1.0)
            nc.sync.dma_start(out=out_d[:], in_=o_sb[:])

    nc.compile()
    return nc


_NC = None


def _get_nc():
    global _NC
    if _NC is None:
        _NC = build_program()
    return _NC


def _host_prep(inputs):
    """Build per-core in_maps from full inputs (weight transforms on host)."""
    f = {k: np.asarray(v, dtype=np.float32) for k, v in inputs.items()}

    # v_i = Wv_w[i] @ att_w[i],  c_i = att_w[i].Wv_b[i] + att_b[i]
    Wv_w, att_w = f["Wv_w"], f["att_w"]
    Wv_b, att_b = f["Wv_b"], f["att_b"]
    vs = [Wv_w[i] @ att_w[i] for i in range(DEPTH)]
    cs = [float(att_w[i] @ Wv_b[i] + att_b[i]) for i in range(DEPTH)]

    wcols = {
        "p_w": f["p_w"], "d_w": f["d_w"],
        "Wv0a": f["Wv0_w"][:H], "Wv0b": f["Wv0_w"][H:],
        "Wv_0": Wv_w[0], "Wv_1": Wv_w[1], "Wv_2": Wv_w[2],
        "vrep_0": np.repeat(vs[0][:, None], 128, 1),
        "vrep_1": np.repeat(vs[1][:, None], 128, 1),
        "vrep_2": np.repeat(vs[2][:, None], 128, 1),
        "W1a": f["W1a_w"],
        "W1b": f["W1b_w"],
        "W2a_0": f["W2a_w"][:, :H], "W2a_1": f["W2a_w"][:, H:],
        "W2b_0": f["W2b_w"][:H], "W2b_1": f["W2b_w"][H:],
        "W3": f["W3_w"],
        "Wu_0": f["Wu_w"][:H], "Wu_1": f["Wu_w"][H:],
        "Wu1a_0": f["Wu1a_w"][:H], "Wu1a_1": f["Wu1a_w"][H:],
        "Wu1b": f["Wu1b_w"],
    }
    wmat = np.concatenate([np.ascontiguousarray(wcols[n]) for n in W_NAMES],
                          axis=1)

    def bc(v):
        v = np.asarray(v, dtype=np.float32).reshape(-1)
        if v.size == 1:
            return np.full((H,), float(v[0]), dtype=np.float32)
        assert v.size == H
        return v

    bvals = {
        "p_b": f["p_b"], "d_b": f["d_b"], "Wv0_b": f["Wv0_b"],
        "Wv_b0": Wv_b[0], "Wv_b1": Wv_b[1], "Wv_b2": Wv_b[2],
        "c0": cs[0], "c1": cs[1], "c2": cs[2],
        "Wu_b": f["Wu_b"], "W1a_b": f["W1a_b"],
        "W1b_bN": f["W1b_b"] * np.float32(N),
        "W2a_b0": f["W2a_b"][:H], "W2a_b1": f["W2a_b"][H:],
        "W2b_b": f["W2b_b"], "W3_b": f["W3_b"],
        "Wu1a_b": f["Wu1a_b"], "Wu1b_b": f["Wu1b_b"],
        "W5_b": f["W5_b"], "W5_w": f["W5_w"][:, 0],
    }
    bmat = np.stack([bc(bvals[n]) for n in B_NAMES], axis=1)
    bmat = np.ascontiguousarray(bmat, dtype=np.float32)

    pf, df = f["protein_features"], f["drug_features"]
    pm, dm = f["pu_mask"], f["du_mask"]
    in_maps = []
    for b in range(B):
        in_maps.append({
            "pfT": np.ascontiguousarray(pf[b].T),
            "dfT": np.ascontiguousarray(df[b].T),
            "pmask": np.ascontiguousarray(
                np.broadcast_to(pm[b][None, :], (H, NP))),
            "dmask": np.ascontiguousarray(
                np.broadcast_to(dm[b][None, :], (H, ND))),
            "wmat": wmat,
            "bmat": bmat,
        })
    return in_maps


def kernel(**inputs) -> np.ndarray:
    nc = _get_nc()
    in_maps = _host_prep(inputs)
    res = run_bass_kernel_spmd(nc, in_maps, list(range(B)))
    out = np.concatenate([res.results[b]["out"] for b in range(B)], axis=0)
    return out.astype(np.float32).reshape(B, 1)


# revision 25
# speedup vs baseline: 1.2212x; 1.0067x over previous
"""Trainium2 Bass kernel for nn_Affinity (gnn_message_passing).

Math (per batch element b, H=128, NP=512, ND=64, DEPTH=3, N=NP*ND=32768):
  drug_feat = lrelu(drug @ d_w + d_b, .1) * du_mask ; u_d = drug_feat.sum(0)
  prot_feat = lrelu(prot @ p_w + p_b, .1) * pu_mask ; u_p = prot_feat.sum(0)
  mij[p,d]  = lrelu(prot_feat[p] @ Wv0a + drug_feat[d] @ Wv0b + Wv0_b, .1)
  m_u       = lrelu([u_d;u_p] @ Wu_w + Wu_b, .01)
  for i in 0..2:   (per pairwise row m)
    t = m @ Wv_i + Wv_b_i ; alpha = m . v_i + c_i    (v_i = Wv_i@att_i,
    g = alpha * t ; m += g                            c_i = Wv_b_i.att_i+att_b_i)
  w1in = m + g   (= m_prev + 2 g_last)
  s = sum_rows relu(w1in @ W1a + W1a_b)
  global1 = s @ W1b + N*W1b_b ; ... small MLP tail ... -> scalar per batch

Strategy: data-parallel over batch, 1 element per NeuronCore, no collectives.
Everything H-major on chip: tensors stored [H=128 partitions, rows free].
The big [N,128] intermediate lives only in SBUF, tiled FD rows at a time.
Matmuls in fp32r (full PE rate at FD>=256, ~1.5e-4 rel err), rest fp32.
"""

import os
import sys

if "/opt/trn_rl_repo" not in sys.path:
    sys.path.insert(0, "/opt/trn_rl_repo")

import numpy as np
from contextlib import ExitStack

import concourse.bass as bass
import concourse.tile as tile
from concourse import bacc, mybir
from concourse.bass_utils import run_bass_kernel_spmd

F32 = mybir.dt.float32
F32R = mybir.dt.float32r
AF = mybir.ActivationFunctionType
OP = mybir.AluOpType

B, NP, ND, H, DEPTH = 8, 512, 64, 128, 3
N = NP * ND
FD = 512           # rows per macro tile (TD=FD//NP d-values per tile)
TD = FD // NP      # d-values per macro tile
NT = N // FD       # macro tiles per core

# weight slot order in the packed [128, 128*NW] weight matrix
W_NAMES = [
    "p_w", "d_w", "Wv0a", "Wv0b", "Wv_0", "Wv_1", "Wv_2",
    "vrep_0", "vrep_1", "vrep_2", "W1a",
    "W1b", "W2a_0", "W2a_1", "W2b_0", "W2b_1", "W3",
    "Wu_0", "Wu_1", "Wu1a_0", "Wu1a_1", "Wu1b",
]
WIDX = {n: i for i, n in enumerate(W_NAMES)}
NW = len(W_NAMES)

B_NAMES = [
    "p_b", "d_b", "Wv0_b", "Wv_b0", "Wv_b1", "Wv_b2", "c0", "c1", "c2",
    "Wu_b", "W1a_b", "W1b_bN", "W2a_b0", "W2a_b1", "W2b_b", "W3_b",
    "Wu1a_b", "Wu1b_b", "W5_b", "W5_w",
]
BIDX = {n: i for i, n in enumerate(B_NAMES)}
NB = len(B_NAMES)


def build_program():
    import os as _os
    nc = bacc.Bacc("TRN2", target_bir_lowering=False, debug=False, num_devices=8)

    pfT_d = nc.dram_tensor("pfT", [H, NP], F32, kind="ExternalInput")
    dfT_d = nc.dram_tensor("dfT", [H, ND], F32, kind="ExternalInput")
    pmask_d = nc.dram_tensor("pmask", [H, NP], F32, kind="ExternalInput")
    dmask_d = nc.dram_tensor("dmask", [H, ND], F32, kind="ExternalInput")
    wmat_d = nc.dram_tensor("wmat", [H, 128 * NW], F32, kind="ExternalInput")
    bmat_d = nc.dram_tensor("bmat", [H, NB], F32, kind="ExternalInput")
    out_d = nc.dram_tensor("out", [1, 1], F32, kind="ExternalOutput")

    def w(ap, name):
        i = WIDX[name]
        return ap[:, i * 128:(i + 1) * 128]

    def bcol(ap, name):
        i = BIDX[name]
        return ap[:, i:i + 1]

    with tile.TileContext(nc) as tc, ExitStack() as ctx:
        sbw = ctx.enter_context(tc.tile_pool(name="sbw", bufs=1))

        wsb = sbw.tile([H, 128 * NW], F32, tag="wsb")
        for c0 in range(0, 128 * NW, 128 * 6):
            c1 = min(c0 + 128 * 6, 128 * NW)
            nc.sync.dma_start(out=wsb[:, c0:c1], in_=wmat_d[:, c0:c1])
        bsb = sbw.tile([H, NB], F32, tag="bsb")
        nc.sync.dma_start(out=bsb[:], in_=bmat_d[:])
        pfT = sbw.tile([H, NP], F32, tag="pfT")
        nc.sync.dma_start(out=pfT[:], in_=pfT_d[:])
        dfT = sbw.tile([H, ND], F32, tag="dfT")
        nc.sync.dma_start(out=dfT[:], in_=dfT_d[:])
        pmask = sbw.tile([H, NP], F32, tag="pmask")
        nc.sync.dma_start(out=pmask[:], in_=pmask_d[:])
        dmask = sbw.tile([H, ND], F32, tag="dmask")
        nc.sync.dma_start(out=dmask[:], in_=dmask_d[:])

        # fp32r copies of the per-row-GEMM weights
        wr = {}
        for name in ["Wv_0", "Wv_1", "Wv_2", "vrep_0", "vrep_1", "vrep_2",
                     "W1a"]:
            t = sbw.tile([H, 128], F32R, tag=f"r_{name}")
            nc.vector.tensor_copy(t[:], w(wsb, name))
            wr[name] = t

        # ---- startup: projections, u_d/u_p, m_u, prot/drug proj ----
        ppT = sbw.tile([H, NP], F32, tag="ppT")
        dpT = sbw.tile([H, ND], F32, tag="dpT")
        u_p = sbw.tile([H, 1], F32, tag="u_p")
        u_d = sbw.tile([H, 1], F32, tag="u_d")
        mu_t = sbw.tile([H, 1], F32, tag="mu_t")
        acc = sbw.tile([H, NT], F32, tag="acc")

        with tc.tile_pool(name="ps0", bufs=2, space="PSUM") as ps0:
            # prot: feat = prelu(p_w.T @ pfT + p_b); masked + u_p
            ps_pf = ps0.tile([H, NP], F32, tag="ps0")
            nc.tensor.matmul(ps_pf[:], w(wsb, "p_w"), pfT[:], start=True, stop=True)
            pf_act = sbw.tile([H, NP], F32, tag="pf_act")
            nc.scalar.activation(pf_act[:], ps_pf[:], AF.Prelu,
                                 bias=bcol(bsb, "p_b"), scale=1.0, alpha=0.1)
            pf_m = sbw.tile([H, NP], F32, tag="pf_m")
            nc.vector.scalar_tensor_tensor(pf_m[:], pf_act[:], 1.0, pmask[:],
                                           op0=OP.mult, op1=OP.mult,
                                           accum_out=u_p[:])
            # drug
            ps_df = ps0.tile([H, ND], F32, tag="ps0")
            nc.tensor.matmul(ps_df[:], w(wsb, "d_w"), dfT[:], start=True, stop=True)
            df_act = sbw.tile([H, ND], F32, tag="df_act")
            nc.scalar.activation(df_act[:], ps_df[:], AF.Prelu,
                                 bias=bcol(bsb, "d_b"), scale=1.0, alpha=0.1)
            df_m = sbw.tile([H, ND], F32, tag="df_m")
            nc.vector.scalar_tensor_tensor(df_m[:], df_act[:], 1.0, dmask[:],
                                           op0=OP.mult, op1=OP.mult,
                                           accum_out=u_d[:])
            # pairwise projections (Wv0_b folded into prot side)
            ps_pp = ps0.tile([H, NP], F32, tag="ps0")
            nc.tensor.matmul(ps_pp[:], w(wsb, "Wv0a"), pf_m[:], start=True, stop=True)
            nc.scalar.activation(ppT[:], ps_pp[:], AF.Identity,
                                 bias=bcol(bsb, "Wv0_b"), scale=1.0)
            ps_dp = ps0.tile([H, ND], F32, tag="ps0")
            nc.tensor.matmul(ps_dp[:], w(wsb, "Wv0b"), df_m[:], start=True, stop=True)
            nc.scalar.copy(dpT[:], ps_dp[:])
            # m_u = prelu(Wu.T @ [u_d;u_p] + Wu_b, .01)
            ps_mu = ps0.tile([H, 1], F32, tag="ps0s")
            nc.tensor.matmul(ps_mu[:], w(wsb, "Wu_0"), u_d[:], start=True, stop=False)
            nc.tensor.matmul(ps_mu[:], w(wsb, "Wu_1"), u_p[:], start=False, stop=True)
            nc.scalar.activation(mu_t[:], ps_mu[:], AF.Prelu,
                                 bias=bcol(bsb, "Wu_b"), scale=1.0, alpha=0.01)

        # ---- main loop over macro tiles (software-pipelined emission) ----
        # Stages per tile t:
        #   S0: construct m0        S1/S2/S3: depth 0/1/2        S4: W1a+relu
        # Emitted deepest-stage-first per step so each engine's instruction
        # stream interleaves stages of staggered tiles (no cross-tile stalls).
        with tc.tile_pool(name="sbm", bufs=16) as sbm, \
             tc.tile_pool(name="sbg", bufs=12) as sbg, \
             tc.tile_pool(name="pst", bufs=3, space="PSUM") as pst, \
             tc.tile_pool(name="psa", bufs=3, space="PSUM") as psa, \
             tc.tile_pool(name="psw", bufs=2, space="PSUM") as psw:
            NTL = NT
            # test.py wraps the main loop in an on-device For_i for timing
            reps = int(_os.environ.get("K_REPS", "0"))
            mset, g2set = {}, {}

            def s0(t):
                m = sbm.tile([H, FD], F32R, tag="m")
                nc.scalar.activation(m[:], ppT[:], AF.Prelu,
                                     bias=dpT[:, t:t + 1], scale=1.0, alpha=0.1)
                mset[t] = m

            def depth(t, i):
                m = mset[t]
                pa = psa.tile([H, FD], F32, tag="pa")
                pt = pst.tile([H, FD], F32, tag="pt")
                nc.tensor.matmul(pa[:], wr[f"vrep_{i}"][:], m[:],
                                 start=True, stop=True)
                nc.tensor.matmul(pt[:], wr[f"Wv_{i}"][:], m[:],
                                 start=True, stop=True)
                al = sbg.tile([H, FD], F32, tag="al")
                scl = 2.0 if i == 2 else 1.0  # fold g2' = 2*g2 into alpha2
                if (3 * t + i) % 4 == 0:
                    if i == 2:
                        nc.vector.tensor_scalar(al[:], pa[:],
                                                bcol(bsb, f"c{i}"), 2.0,
                                                op0=OP.add, op1=OP.mult)
                    else:
                        nc.vector.tensor_scalar_add(al[:], pa[:],
                                                    bcol(bsb, f"c{i}"))
                else:
                    nc.scalar.activation(al[:], pa[:], AF.Identity,
                                         bias=bcol(bsb, f"c{i}"), scale=scl)
                g = sbg.tile([H, FD], F32R if i == 2 else F32, tag="g")
                nc.vector.scalar_tensor_tensor(g[:], pt[:],
                                               bcol(bsb, f"Wv_b{i}"), al[:],
                                               op0=OP.add, op1=OP.mult)
                if i < 2:
                    m2 = sbm.tile([H, FD], F32R, tag="m")
                    nc.gpsimd.tensor_tensor(m2[:], mset.pop(t)[:].bitcast(F32),
                                            g[:], op=OP.add)
                    mset[t] = m2
                else:
                    g2set[t] = g

            def s4(t):
                m2, g2 = mset.pop(t), g2set.pop(t)
                pw = psw.tile([H, FD], F32, tag="pw")
                nc.tensor.matmul(pw[:], wr["W1a"][:], m2[:],
                                 start=True, stop=False)
                nc.tensor.matmul(pw[:], wr["W1a"][:], g2[:],
                                 start=False, stop=True)
                scr = sbg.tile([H, FD], F32, tag="scr")
                nc.scalar.activation(scr[:], pw[:], AF.Relu,
                                     bias=bcol(bsb, "W1a_b"), scale=1.0,
                                     accum_out=acc[:, t:t + 1])

            LAG = 3

            def main_loop():
                for step in range(NTL + 4 * LAG):
                    if step >= 4 * LAG and step - 4 * LAG < NTL:
                        s4(step - 4 * LAG)
                    if step >= 3 * LAG and step - 3 * LAG < NTL:
                        depth(step - 3 * LAG, 2)
                    if step >= 2 * LAG and step - 2 * LAG < NTL:
                        depth(step - 2 * LAG, 1)
                    if step >= LAG and step - LAG < NTL:
                        depth(step - LAG, 0)
                    if step < NTL:
                        s0(step)

            if reps:
                with tc.For_i(0, reps, 1,
                              hint_engines=(mybir.EngineType.PE,
                                            mybir.EngineType.Activation,
                                            mybir.EngineType.DVE,
                                            mybir.EngineType.Pool,
                                            mybir.EngineType.SP)):
                    main_loop()
            else:
                main_loop()


        # ---- tail MLP (tiny) ----
        with tc.tile_pool(name="pse", bufs=2, space="PSUM") as pse, \
             tc.tile_pool(name="sbe", bufs=1) as sbe:
            s_t = sbe.tile([H, 1], F32, tag="s")
            nc.vector.tensor_reduce(s_t[:], acc[:], axis=mybir.AxisListType.X,
                                    op=OP.add)
            p1 = pse.tile([H, 1], F32, tag="pse")
            nc.tensor.matmul(p1[:], w(wsb, "W1b"), s_t[:], start=True, stop=True)
            t1 = sbe.tile([H, 1], F32, tag="t1")
            nc.scalar.activation(t1[:], p1[:], AF.Identity,
                                 bias=bcol(bsb, "W1b_bN"), scale=1.0)
            p2a = pse.tile([H, 2], F32, tag="pse")
            nc.tensor.matmul(p2a[:, 0:1], w(wsb, "W2a_0"), t1[:], start=True, stop=True)
            nc.tensor.matmul(p2a[:, 1:2], w(wsb, "W2a_1"), t1[:], start=True, stop=True)
            t2a = sbe.tile([H, 2], F32, tag="t2a")
            nc.scalar.activation(t2a[:, 0:1], p2a[:, 0:1], AF.Prelu,
                                 bias=bcol(bsb, "W2a_b0"), scale=1.0, alpha=0.1)
            nc.scalar.activation(t2a[:, 1:2], p2a[:, 1:2], AF.Prelu,
                                 bias=bcol(bsb, "W2a_b1"), scale=1.0, alpha=0.1)
            p2 = pse.tile([H, 1], F32, tag="pse")
            nc.tensor.matmul(p2[:], w(wsb, "W2b_0"), t2a[:, 0:1], start=True, stop=False)
            nc.tensor.matmul(p2[:], w(wsb, "W2b_1"), t2a[:, 1:2], start=False, stop=True)
            t2 = sbe.tile([H, 1], F32, tag="t2")
            nc.scalar.activation(t2[:], p2[:], AF.Identity,
                                 bias=bcol(bsb, "W2b_b"), scale=1.0)
            p3 = pse.tile([H, 1], F32, tag="pse")
            nc.tensor.matmul(p3[:], w(wsb, "W3"), t2[:], start=True, stop=True)
            t3 = sbe.tile([H, 1], F32, tag="t3")
            nc.scalar.activation(t3[:], p3[:], AF.Prelu,
                                 bias=bcol(bsb, "W3_b"), scale=1.0, alpha=0.1)
            p4 = pse.tile([H, 1], F32, tag="pse")
            nc.tensor.matmul(p4[:], w(wsb, "Wu1a_0"), mu_t[:], start=True, stop=False)
            nc.tensor.matmul(p4[:], w(wsb, "Wu1a_1"), t3[:], start=False, stop=True)
            t4 = sbe.tile([H, 1], F32, tag="t4")
            nc.scalar.activation(t4[:], p4[:], AF.Prelu,
                                 bias=bcol(bsb, "Wu1a_b"), scale=1.0, alpha=0.1)
            p5 = pse.tile([H, 1], F32, tag="pse")
            nc.tensor.matmul(p5[:], w(wsb, "Wu1b"), t4[:], start=True, stop=True)
            t5 = sbe.tile([H, 1], F32, tag="t5")
            nc.scalar.activation(t5[:], p5[:], AF.Identity,
                                 bias=bcol(bsb, "Wu1b_b"), scale=1.0)
            p6 = pse.tile([1, 1], F32, tag="pse")
            nc.tensor.matmul(p6[:], bcol(bsb, "W5_w"), t5[:], start=True, stop=True)
            o_sb = sbe.tile([1, 1], F32, tag="o")
            nc.scalar.activation(o_sb[:], p6[:], AF.Identity,
                                 bias=bsb[0:1, BIDX["W5_b"]:BIDX["W5_b"] + 1],
                                 scale=1.0)
            nc.sync.dma_start(out=out_d[:], in_=o_sb[:])

    nc.compile()
    return nc


_NC = None


def _get_nc():
    global _NC
    if _NC is None:
        _NC = build_program()
    return _NC


def _host_prep(inputs):
    """Build per-core in_maps from full inputs (weight transforms on host)."""
    f = {k: np.asarray(v, dtype=np.float32) for k, v in inputs.items()}

    # v_i = Wv_w[i] @ att_w[i],  c_i = att_w[i].Wv_b[i] + att_b[i]
    Wv_w, att_w = f["Wv_w"], f["att_w"]
    Wv_b, att_b = f["Wv_b"], f["att_b"]
    vs = [Wv_w[i] @ att_w[i] for i in range(DEPTH)]
    cs = [float(att_w[i] @ Wv_b[i] + att_b[i]) for i in range(DEPTH)]

    wcols = {
        "p_w": f["p_w"], "d_w": f["d_w"],
        "Wv0a": f["Wv0_w"][:H], "Wv0b": f["Wv0_w"][H:],
        "Wv_0": Wv_w[0], "Wv_1": Wv_w[1], "Wv_2": Wv_w[2],
        "vrep_0": np.repeat(vs[0][:, None], 128, 1),
        "vrep_1": np.repeat(vs[1][:, None], 128, 1),
        "vrep_2": np.repeat(vs[2][:, None], 128, 1),
        "W1a": f["W1a_w"],
        "W1b": f["W1b_w"],
        "W2a_0": f["W2a_w"][:, :H], "W2a_1": f["W2a_w"][:, H:],
        "W2b_0": f["W2b_w"][:H], "W2b_1": f["W2b_w"][H:],
        "W3": f["W3_w"],
        "Wu_0": f["Wu_w"][:H], "Wu_1": f["Wu_w"][H:],
        "Wu1a_0": f["Wu1a_w"][:H], "Wu1a_1": f["Wu1a_w"][H:],
        "Wu1b": f["Wu1b_w"],
    }
    wmat = np.concatenate([np.ascontiguousarray(wcols[n]) for n in W_NAMES],
                          axis=1)

    def bc(v):
        v = np.asarray(v, dtype=np.float32).reshape(-1)
        if v.size == 1:
            return np.full((H,), float(v[0]), dtype=np.float32)
        assert v.size == H
        return v

    bvals = {
        "p_b": f["p_b"], "d_b": f["d_b"], "Wv0_b": f["Wv0_b"],
        "Wv_b0": Wv_b[0], "Wv_b1": Wv_b[1], "Wv_b2": Wv_b[2],
        "c0": cs[0], "c1": cs[1], "c2": cs[2],
        "Wu_b": f["Wu_b"], "W1a_b": f["W1a_b"],
        "W1b_bN": f["W1b_b"] * np.float32(N),
        "W2a_b0": f["W2a_b"][:H], "W2a_b1": f["W2a_b"][H:],
        "W2b_b": f["W2b_b"], "W3_b": f["W3_b"],
        "Wu1a_b": f["Wu1a_b"], "Wu1b_b": f["Wu1b_b"],
        "W5_b": f["W5_b"], "W5_w": f["W5_w"][:, 0],
    }
    bmat = np.stack([bc(bvals[n]) for n in B_NAMES], axis=1)
    bmat = np.ascontiguousarray(bmat, dtype=np.float32)

    pf, df = f["protein_features"], f["drug_features"]
    pm, dm = f["pu_mask"], f["du_mask"]
    in_maps = []
    for b in range(B):
        in_maps.append({
            "pfT": np.ascontiguousarray(pf[b].T),
            "dfT": np.ascontiguousarray(df[b].T),
            "pmask": np.ascontiguousarray(
                np.broadcast_to(pm[b][None, :], (H, NP))),
            "dmask": np.ascontiguousarray(
                np.broadcast_to(dm[b][None, :], (H, ND))),
            "wmat": wmat,
            "bmat": bmat,
        })
    return in_maps


def kernel(**inputs) -> np.ndarray:
    nc = _get_nc()
    in_maps = _host_prep(inputs)
    res = run_bass_kernel_spmd(nc, in_maps, list(range(B)))
    out = np.concatenate([res.results[b]["out"] for b in range(B)], axis=0)
    return out.astype(np.float32).reshape(B, 1)
